# revision 6
# baseline (speedup 1.0000x reference)
"""Trainium2 Bass kernel for nn_LogicLayer (soft 16-gate logic layer).

Computation:
    w      = softmax(weights / TAU, axis=-1)            # [O, 16]
    coeffs = w @ GATE_COEFFS                            # [O, 4]
    a      = x[:, idx_a]; b = x[:, idx_b]               # [B, O] gathers
    out    = c0 + ca*a + cb*b + cab*(a*b)               # [B, O]

Strategy (8 NeuronCores = 2 batch shards x 4 out_dim shards):
  - Host transposes x -> xT [IN_DIM, B]; each core holds a batch-half
    xT_half [IN_DIM, 1024] f32 so the irregular gather along in_dim becomes a
    row gather of contiguous 4KB rows (measured ~326 GB/s marginal via
    SWDGE indirect DMA; 1KB rows are per-instruction-overhead bound).
  - Core (i,j) computes out neurons j*8192..(j+1)*8192 for batch half i.
  - Per 128-neuron chunk: two indirect_dma_start gathers (128 rows x 4KB),
    then t = ca*a + c0 (ACT), u = cab*a + cb (DVE), u *= b, t += u (DVE),
    all full-tile [128, 1024] ops with per-partition coefficient scalars,
    then one contiguous 512KB HWDGE store to outT_q [8192, 1024].
  - Gate coefficients are computed on device from the raw weights
    (exp -> block reduce -> reciprocal -> fold with GATE_COEFFS).
  - Host reassembles the 8 outT shards into [B, O].
"""

import os
import sys
import types

import numpy as np


def _install_ntff_hook():
    """Provide antenv.axon_hooks (absent in this container) so
    run_bass_kernel_spmd(trace=True) can capture NTFF profiles via the
    axon PJRT plugin's C API."""
    if "antenv.axon_hooks" in sys.modules:
        return
    try:
        import antenv
        from trn_agent_boot.trn_boot import _ntff_profile_via_ctypes
    except ImportError:
        return
    mod = types.ModuleType("antenv.axon_hooks")
    _state = {"hook": None}
    mod.set_axon_ntff_profile_hook = lambda h: _state.__setitem__("hook", h)
    mod.get_axon_ntff_profile_hook = lambda: _state["hook"]
    sys.modules["antenv.axon_hooks"] = mod
    antenv.axon_hooks = mod
    try:
        hook = _ntff_profile_via_ctypes("/opt/axon/libaxon_pjrt.so")
    except OSError:
        hook = None
    mod.set_axon_ntff_profile_hook(hook)


_install_ntff_hook()

import concourse.bass as bass
import concourse.bacc as bacc
import concourse.mybir as mybir
import concourse.tile as tile
from concourse.bass_utils import run_bass_kernel_spmd

F32 = mybir.dt.float32
I32 = mybir.dt.int32

N_CORES = 8
B = 2048
IN_DIM = 32768
OUT_DIM = 32768
TAU = 1.0
P = 128

B_SHARDS = 2
O_SHARDS = 4
B_SH = B // B_SHARDS           # 1024 batch rows per core
O_SH = OUT_DIM // O_SHARDS     # 8192 out neurons per core
NCH = O_SH // P                # 64 chunks (one gather pair each)

_GATE_COEFFS = np.array([
    [0.,  0.,  0.,  0.],
    [0.,  0.,  0.,  1.],
    [0.,  1.,  0., -1.],
    [0.,  1.,  0.,  0.],
    [0.,  0.,  1., -1.],
    [0.,  0.,  1.,  0.],
    [0.,  1.,  1., -2.],
    [0.,  1.,  1., -1.],
    [1., -1., -1.,  1.],
    [1., -1., -1.,  2.],
    [1.,  0., -1.,  0.],
    [1.,  0., -1.,  1.],
    [1., -1.,  0.,  0.],
    [1., -1.,  0.,  1.],
    [1.,  0.,  0., -1.],
    [1.,  0.,  0.,  0.],
], dtype=np.float32)

LAST_RESULTS = None  # BassKernelResults of the most recent run (for test.py)

_module_cache = {}


def build_module():
    if "nc" in _module_cache:
        return _module_cache["nc"]

    nc = bacc.Bacc()

    xT = nc.declare_dram_parameter("xT", [IN_DIM, B_SH], F32, isOutput=False)
    idxa = nc.declare_dram_parameter("idxa", [P, NCH], I32, isOutput=False)
    idxb = nc.declare_dram_parameter("idxb", [P, NCH], I32, isOutput=False)
    wpre = nc.declare_dram_parameter("wpre", [P, NCH * 16], F32, isOutput=False)
    gmat = nc.declare_dram_parameter("gmat", [P, 4 * 16], F32, isOutput=False)
    outT = nc.declare_dram_parameter("outT", [O_SH, B_SH], F32, isOutput=True)

    Exp = mybir.ActivationFunctionType.Exp
    Identity = mybir.ActivationFunctionType.Identity
    mult = mybir.AluOpType.mult
    add = mybir.AluOpType.add
    X = mybir.AxisListType.X

    with tile.TileContext(nc) as tc:
        with (
            tc.tile_pool(name="const", bufs=1) as constp,
            tc.tile_pool(name="gath", bufs=6) as gathp,
            tc.tile_pool(name="work", bufs=3) as workp,
        ):
            # ---- load metadata ----
            ia_sb = constp.tile([P, NCH], I32, tag="ia")
            ib_sb = constp.tile([P, NCH], I32, tag="ib")
            g_sb = constp.tile([P, 4 * 16], F32, tag="g")
            nc.sync.dma_start(out=ia_sb[:], in_=idxa[:])
            nc.sync.dma_start(out=ib_sb[:], in_=idxb[:])
            nc.sync.dma_start(out=g_sb[:], in_=gmat[:])

            # ---- coefficients: softmax fold (one-time, small) ----
            w_sb = gathp.tile([P, NCH * 16], F32, tag="a")
            wexp = gathp.tile([P, NCH * 16], F32, tag="b")
            tmp = workp.tile([P, NCH * 16], F32, tag="u")
            nc.sync.dma_start(out=w_sb[:], in_=wpre[:])
            nc.scalar.activation(wexp[:], w_sb[:], Exp, scale=1.0 / TAU)
            ssum = constp.tile([P, NCH], F32, tag="ssum")
            wexp3 = wexp[:].rearrange("p (c g) -> p c g", g=16)
            nc.vector.reduce_sum(ssum[:], wexp3, axis=X)
            rinv = constp.tile([P, NCH], F32, tag="rinv")
            nc.vector.reciprocal(rinv[:], ssum[:])

            # coeff[:, j*NCH:(j+1)*NCH] = (wexp . G[:, j]) * rinv
            coeff = constp.tile([P, 4 * NCH], F32, tag="coeff")
            g3 = g_sb[:].rearrange("p (j g) -> p j g", g=16)
            tmp3 = tmp[:].rearrange("p (c g) -> p c g", g=16)
            for j in range(4):
                gj = g3[:, j : j + 1, :].to_broadcast([P, NCH, 16])
                nc.vector.tensor_mul(tmp3, wexp3, gj)
                cj = coeff[:, j * NCH : (j + 1) * NCH]
                nc.vector.reduce_sum(cj, tmp3, axis=X)
                nc.vector.tensor_mul(cj, cj, rinv[:])
            c0 = coeff[:, 0 * NCH : 1 * NCH]
            ca = coeff[:, 1 * NCH : 2 * NCH]
            cb = coeff[:, 2 * NCH : 3 * NCH]
            cab = coeff[:, 3 * NCH : 4 * NCH]

            # ---- main loop: gather + affine combine + store ----
            for c in range(NCH):
                a = gathp.tile([P, B_SH], F32, tag="a")
                b = gathp.tile([P, B_SH], F32, tag="b")
                u = workp.tile([P, B_SH], F32, tag="u")
                t = workp.tile([P, B_SH], F32, tag="t")

                nc.gpsimd.indirect_dma_start(
                    out=a[:],
                    out_offset=None,
                    in_=xT[:],
                    in_offset=bass.IndirectOffsetOnAxis(
                        ap=ia_sb[:, c : c + 1], axis=0
                    ),
                )
                nc.gpsimd.indirect_dma_start(
                    out=b[:],
                    out_offset=None,
                    in_=xT[:],
                    in_offset=bass.IndirectOffsetOnAxis(
                        ap=ib_sb[:, c : c + 1], axis=0
                    ),
                )

                # t = ca*a + c0   (ACT, per-partition scale/bias)
                nc.scalar.activation(
                    t[:], a[:], Identity,
                    bias=c0[:, c : c + 1],
                    scale=ca[:, c : c + 1],
                )
                # u = cab*a + cb  (DVE, two per-partition scalars)
                nc.vector.tensor_scalar(
                    u[:], a[:],
                    cab[:, c : c + 1],
                    cb[:, c : c + 1],
                    mult, add,
                )
                nc.vector.tensor_mul(u[:], u[:], b[:])
                nc.vector.tensor_add(t[:], t[:], u[:])

                nc.sync.dma_start(out=outT[c * P : (c + 1) * P, :], in_=t[:])

    _module_cache["nc"] = nc
    return nc


def _prep_inputs(x, idx_a, idx_b, weights):
    """Host-side shard/layout prep. Returns per-core input maps.

    Core k = i*O_SHARDS + j handles batch half i, out-neuron quarter j.
    """
    x = np.ascontiguousarray(np.asarray(x, dtype=np.float32))
    idx_a = np.asarray(idx_a).astype(np.int64)
    idx_b = np.asarray(idx_b).astype(np.int64)
    weights = np.ascontiguousarray(np.asarray(weights, dtype=np.float32))

    gmat = np.ascontiguousarray(
        np.broadcast_to(_GATE_COEFFS.T.reshape(1, 4, 16), (P, 4, 16)).reshape(P, 64)
    ).astype(np.float32)

    xT_halves = [
        np.ascontiguousarray(x[i * B_SH : (i + 1) * B_SH].T) for i in range(B_SHARDS)
    ]

    per_quarter = []
    for j in range(O_SHARDS):
        sl = slice(j * O_SH, (j + 1) * O_SH)
        ia = np.ascontiguousarray(
            idx_a[sl].reshape(NCH, P).T.astype(np.int32)
        )
        ib = np.ascontiguousarray(
            idx_b[sl].reshape(NCH, P).T.astype(np.int32)
        )
        wq = np.ascontiguousarray(
            weights[sl].reshape(NCH, P, 16).transpose(1, 0, 2).reshape(P, NCH * 16)
        )
        per_quarter.append((ia, ib, wq))

    in_maps = []
    for k in range(N_CORES):
        i, j = divmod(k, O_SHARDS)
        ia, ib, wq = per_quarter[j]
        in_maps.append(
            {"xT": xT_halves[i], "idxa": ia, "idxb": ib, "wpre": wq, "gmat": gmat}
        )
    return in_maps


def kernel(x, idx_a, idx_b, weights):
    global LAST_RESULTS
    nc = build_module()
    if not nc.is_finalized():
        nc.finalize()
    in_maps = _prep_inputs(x, idx_a, idx_b, weights)
    res = run_bass_kernel_spmd(
        nc,
        in_maps,
        core_ids=list(range(N_CORES)),
        trace=bool(int(os.environ.get("KERNEL_TRACE", "0"))),
    )
    LAST_RESULTS = res
    out = np.empty((B, OUT_DIM), dtype=np.float32)
    for k in range(N_CORES):
        i, j = divmod(k, O_SHARDS)
        out[i * B_SH : (i + 1) * B_SH, j * O_SH : (j + 1) * O_SH] = (
            res.results[k]["outT"].T
        )
    return out
